# revision 11
# baseline (speedup 1.0000x reference)
"""ConcatCritic pair-grid MLP, v9: contiguous permuted loads + lean setup.

All big inputs are loaded CONTIGUOUSLY (128 large DMA descriptors instead
of 512 small ones — the load phase is descriptor-bound): partition p of a
[512, n]-row tensor holds rows 4p..4p+3, i.e. row index = 4p + r.  The
resulting interleave is absorbed as one consistent permutation:
  h = 4c + hb  (w1/b1 rows, A4 partitions, w2T rows)
  j = 4c + jc  (y rows, A4/hy columns, pz/acc partitions)
  k = 4p + r   (w2 rows, pz columns, w3/b2 columns — w3 row pre-permuted)
The permutation cancels in all contractions; the only places it surfaces:
w2 transpose sources are stride-4 column slices, w3 (and b2) rows are
pre-permuted with 4 strided DVE copies, and the host unshard reshapes
out_d[c, jc, i] -> score[i, 4c + jc].

Per-core (64 rows of x, everything else replicated):
  per i:  ACT  A4[:, hb, :] = relu(hyT[hb] + hxbT[:, hb*64+i])   (4 ops, fp16)
          PE   pz[jc][j, k] += A4[:, hb, jc*128:+128].T @ w2T[hb]  (16 matmuls)
          DVE  acc[jc][:, i] = sum_k relu(pz[jc]) * w3bc          (4 fused ops)

The W3 reduction costs zero PE time (fused into the DVE relu). b2 is
zero in this model family; a fallback build adds an exact K=1 matmul
(ones.T @ b2perm) into each psum accumulation when b2 != 0.
"""

import os

import numpy as np

import concourse.bass as bass
import concourse.bacc as bacc
import concourse.mybir as mybir
from concourse import tile
from concourse.masks import make_identity
from concourse.bass_utils import run_bass_kernel_spmd

B = 512
D = 128
H = 512
NCORES = 8
BI = B // NCORES  # 64 rows of x per core
HB = H // 128     # 4 h-blocks
JC = B // 128     # 4 j-chunks
FP = mybir.dt.float32
F16 = mybir.dt.float16

Relu = mybir.ActivationFunctionType.Relu
Identity = mybir.ActivationFunctionType.Identity
Add = mybir.AluOpType.add
Max = mybir.AluOpType.max
Mult = mybir.AluOpType.mult
Bypass = mybir.AluOpType.bypass


def build_v9(b2_nonzero: bool = False) -> bass.Bass:
    nc = bacc.Bacc(
        "TRN2",
        target_bir_lowering=False,
        debug=False,
        enable_asserts=False,
    )

    xs_d = nc.dram_tensor("xs", [BI, D], FP, kind="ExternalInput")
    y_d = nc.dram_tensor("y", [B, D], FP, kind="ExternalInput")
    W1_d = nc.dram_tensor("W1", [H, 2 * D], FP, kind="ExternalInput")
    b1_d = nc.dram_tensor("b1", [H], FP, kind="ExternalInput")
    W2_d = nc.dram_tensor("W2", [H, H], FP, kind="ExternalInput")
    b2_d = nc.dram_tensor("b2", [H], FP, kind="ExternalInput")
    W3_d = nc.dram_tensor("W3", [1, H], FP, kind="ExternalInput")
    b3_d = nc.dram_tensor("b3", [1], FP, kind="ExternalInput")
    # out_d[c, jc, i] = score[i, 4c + jc]; host reshapes at unshard.
    out_d = nc.dram_tensor("out", [128, JC, BI], FP, kind="ExternalOutput")

    with tile.TileContext(nc) as tc:
        with (
            tc.tile_pool(name="consts", bufs=1) as consts,
            tc.tile_pool(name="persist", bufs=1) as persist,
            tc.tile_pool(name="load", bufs=1) as load,
            tc.tile_pool(name="work", bufs=3) as work,
            tc.tile_pool(name="ps", bufs=8, space="PSUM") as ps,
        ):
            ident = consts.tile([128, 128], FP, name="ident")
            make_identity(nc, ident)

            # ------- input DMAs: contiguous loads, one dma_start each -------
            # sync queue: critical-path tensors.  scalar queue: w2 + consts.
            xs_sb = load.tile([BI, D], FP, name="xs_sb")
            nc.sync.dma_start(xs_sb, xs_d[:, :])
            w1_sb = load.tile([128, HB, 2 * D], FP, name="w1_sb")
            nc.sync.dma_start(w1_sb, W1_d[:].rearrange("(p r) d -> p r d", p=128))
            y_sb = load.tile([128, JC, D], FP, name="y_sb")
            nc.sync.dma_start(y_sb, y_d[:].rearrange("(p r) d -> p r d", p=128))
            w2_sb = load.tile([128, HB, H], FP, name="w2_sb")
            nc.scalar.dma_start(w2_sb, W2_d[:].rearrange("(p r) h -> p r h", p=128))
            b1c = consts.tile([128, HB], FP, name="b1c")
            nc.scalar.dma_start(b1c, b1_d[:].rearrange("(p r) -> p r", p=128))
            w3row = consts.tile([1, H], FP, name="w3row")
            b3c = consts.tile([1, 1], FP, name="b3c")
            nc.scalar.dma_start(w3row, W3_d[:, :])
            nc.scalar.dma_start(b3c, b3_d[None, :])
            if b2_nonzero:
                b2row = consts.tile([1, H], F16, name="b2row")
                b2row32 = consts.tile([1, H], FP, name="b2row32")
                nc.scalar.dma_start(b2row32, b2_d[None, :])
                b2p32 = consts.tile([1, H], FP, name="b2p32")
                for r in range(4):
                    nc.vector.tensor_copy(
                        b2p32[:, r * 128 : (r + 1) * 128], b2row32[:, r::4]
                    )
                nc.vector.tensor_copy(b2row, b2p32)
                ones_st = consts.tile([1, 128], F16, name="ones_st")
                nc.vector.memset(ones_st, 1.0)

            ones1 = consts.tile([1, 128], FP, name="ones1")
            nc.vector.memset(ones1, 1.0)

            # ---------------- transposes + mm1 ----------------
            xsT = persist.tile([128, BI], FP, name="xsT")
            t_ps = ps.tile([128, 128], FP, tag="tbank", bufs=3, name="t_ps_x")
            nc.tensor.transpose(t_ps[:, :BI], xs_sb, ident[:BI, :BI])
            nc.vector.tensor_copy(xsT, t_ps[:, :BI])

            # w1xT[hb][d, c] = W1[4c + hb, d] (x half); same for y half
            w1xT = []
            w1yT = []
            for hb in range(HB):
                tx = persist.tile([128, 128], FP, name=f"w1xT{hb}")
                ty = persist.tile([128, 128], FP, name=f"w1yT{hb}")
                px = ps.tile([128, 128], FP, tag="tbank", bufs=3, name=f"t_ps_w1x{hb}")
                nc.tensor.transpose(px, w1_sb[:, hb, :D], ident)
                nc.vector.tensor_copy(tx, px)
                py = ps.tile([128, 128], FP, tag="tbank", bufs=3, name=f"t_ps_w1y{hb}")
                nc.tensor.transpose(py, w1_sb[:, hb, D:], ident)
                nc.vector.tensor_copy(ty, py)
                w1xT.append(tx)
                w1yT.append(ty)

            # yT[d, r*128 + c] = y[4c + r, d]
            yT = persist.tile([128, B], FP, name="yT")
            for jb in range(B // 128):
                pj = ps.tile([128, 128], FP, tag="tbank", bufs=3, name=f"t_ps_y{jb}")
                nc.tensor.transpose(pj, y_sb[:, jb, :], ident)
                nc.vector.tensor_copy(yT[:, jb * 128 : (jb + 1) * 128], pj)

            # hxbT[c, hb*BI + i] = hx[i, 4c+hb] + b1[4c+hb]   (fp32)
            hxbT = persist.tile([128, HB * BI], FP, name="hxbT")
            hyT = [persist.tile([128, B], F16, name=f"hyT{hb}") for hb in range(HB)]
            for hb in range(HB):
                hx_ps = ps.tile([128, BI], FP, tag="tbank", bufs=3, name=f"hx_ps{hb}")
                nc.tensor.matmul(hx_ps, w1xT[hb], xsT, start=True, stop=True)
                nc.scalar.activation(
                    hxbT[:, hb * BI : (hb + 1) * BI],
                    hx_ps,
                    Identity,
                    bias=b1c[:, hb : hb + 1],
                )
                hy_ps = ps.tile([128, B], FP, tag="pz", bufs=4, name=f"hy_ps{hb}")
                nc.tensor.matmul(hy_ps, w1yT[hb], yT, start=True, stop=True)
                nc.scalar.activation(hyT[hb], hy_ps, Identity)

            # w2T[hb][c, r*128 + p] = W2[4p + r, 4c + hb]  (fp16)
            # source columns are the stride-4 h-slice matching A4's h order
            w2T = [persist.tile([128, H], F16, name=f"w2T{hb}") for hb in range(HB)]
            for r in range(HB):
                for hb in range(HB):
                    pw = ps.tile(
                        [128, 128], FP, tag="tbank", bufs=3, name=f"t_ps_w2_{r}_{hb}"
                    )
                    nc.tensor.transpose(pw, w2_sb[:, r, hb::4], ident)
                    dst = w2T[hb][:, r * 128 : (r + 1) * 128]
                    if (r * HB + hb) % 2 == 0:
                        nc.vector.tensor_copy(dst, pw)
                    else:
                        nc.scalar.activation(dst, pw, Identity)

            # permuted w3 row: w3p[0, r*128 + c] = W3[0, 4c + r]
            w3p = consts.tile([1, H], FP, name="w3p")
            for r in range(4):
                nc.vector.tensor_copy(w3p[:, r * 128 : (r + 1) * 128], w3row[:, r::4])

            # w3 broadcast to all 128 partitions: w3bc[j, k'] = w3p[k']
            w3bc_ps = ps.tile([128, B], FP, tag="misc", bufs=1, name="w3bc_ps")
            nc.tensor.matmul(w3bc_ps, ones1, w3p, start=True, stop=True)
            w3bc = consts.tile([128, B], FP, name="w3bc")
            nc.vector.tensor_copy(w3bc, w3bc_ps)

            # accumulator staging: acc[jc][c, i] = score[i, 4c + jc]
            acc = [persist.tile([128, BI], FP, name=f"acc{jc}") for jc in range(JC)]
            dummy = persist.tile([128, B], F16, name="stt_dummy")

            # ---------------- main loop ----------------
            def gen_A(i, A4):
                for hb in range(HB):
                    bias = hxbT[:, hb * BI + i : hb * BI + i + 1]
                    if i < 2 and hb < 2:
                        nc.vector.tensor_scalar(
                            A4[:, hb, :], hyT[hb], bias, 0.0, Add, Max
                        )
                    else:
                        nc.scalar.activation(A4[:, hb, :], hyT[hb], Relu, bias=bias)

            A_bufs = [
                work.tile([128, HB, B], F16, tag="A4", bufs=3, name=f"A4_{p}")
                for p in range(3)
            ]

            gen_A(0, A_bufs[0])
            for i in range(BI):
                A4 = A_bufs[i % 3]
                if i + 1 < BI:
                    gen_A(i + 1, A_bufs[(i + 1) % 3])
                for jc in range(JC):
                    pz = ps.tile(
                        [128, B], FP, tag="pz", bufs=4, name=f"pz{i}_{jc}"
                    )
                    if b2_nonzero:
                        nc.tensor.matmul(pz, ones_st, b2row, start=True, stop=False)
                    for hb in range(HB):
                        nc.tensor.matmul(
                            pz,
                            A4[:, hb, jc * 128 : (jc + 1) * 128],
                            w2T[hb],
                            start=(hb == 0 and not b2_nonzero),
                            stop=(hb == HB - 1),
                        )
                    # acc[jc][:, i] = sum_k relu(pz) * w3
                    nc.vector.scalar_tensor_tensor(
                        dummy,
                        pz,
                        0.0,
                        w3bc,
                        Max,
                        Mult,
                        accum_out=acc[jc][:, i : i + 1],
                    )

            # ---------------- tail: +b3, DMA out (permuted layout) ----------
            # b3 broadcast built late so it stays out of the setup PE stream
            b3_ps = ps.tile([128, 1], FP, tag="misc", bufs=1, name="b3_ps")
            nc.tensor.matmul(b3_ps, ones1, b3c, start=True, stop=True)
            b3bc = consts.tile([128, 1], FP, name="b3bc")
            nc.vector.tensor_copy(b3bc, b3_ps)

            out_q = [nc.sync, nc.scalar, nc.sync, nc.scalar]
            for jc in range(JC):
                outj = persist.tile([128, BI], FP, name=f"outj{jc}")
                nc.vector.tensor_scalar(outj, acc[jc], b3bc, 0.0, Add, Bypass)
                out_q[jc].dma_start(out_d[:, jc, :], outj)

    nc.compile()
    return nc


_BUILT: dict[str, bass.Bass] = {}


def _get_nc(key: str) -> bass.Bass:
    if key not in _BUILT:
        _BUILT[key] = build_v9(b2_nonzero=(key == "b2"))
    return _BUILT[key]


def run(inputs: dict, variant: str | None = None, trace: bool = False):
    x = np.ascontiguousarray(np.asarray(inputs["x"], dtype=np.float32))
    y = np.ascontiguousarray(np.asarray(inputs["y"], dtype=np.float32))
    W1 = np.ascontiguousarray(np.asarray(inputs["W1"], dtype=np.float32))
    b1 = np.ascontiguousarray(np.asarray(inputs["b1"], dtype=np.float32))
    W2 = np.ascontiguousarray(np.asarray(inputs["W2"], dtype=np.float32))
    b2 = np.ascontiguousarray(np.asarray(inputs["b2"], dtype=np.float32))
    W3 = np.ascontiguousarray(np.asarray(inputs["W3"], dtype=np.float32))
    b3 = np.ascontiguousarray(np.asarray(inputs["b3"], dtype=np.float32))
    nc = _get_nc("b2" if np.any(b2) else "z")
    in_maps = []
    for c in range(NCORES):
        in_maps.append(
            {
                "xs": np.ascontiguousarray(x[c * BI : (c + 1) * BI]),
                "y": y,
                "W1": W1,
                "b1": b1,
                "W2": W2,
                "b2": b2,
                "W3": W3,
                "b3": b3,
            }
        )
    res = run_bass_kernel_spmd(nc, in_maps, core_ids=list(range(NCORES)), trace=trace)
    # r["out"][c, jc, i] = score[i, 4c + jc] -> transpose to [i, c, jc] and
    # flatten: column index c*4 + jc = j.
    out = np.concatenate(
        [r["out"].transpose(2, 0, 1).reshape(BI, B) for r in res.results], axis=0
    )
    return np.ascontiguousarray(out), res


def kernel(**inputs) -> np.ndarray:
    out, _ = run(inputs)
    return out
